# revision 26
# baseline (speedup 1.0000x reference)
"""Trainium2 Bass kernel for quantized ConvBlock (fake-quant -> conv3x3 -> BN -> relu6 fake-quant).

Strategy
--------
Data-parallel over batch: 32 images -> 4 per NeuronCore x 8 cores.

Math: the reference fake-quantizes activations to the 256-level grid
k*(6/255), k in [0,255], and weights to m*(s/127), m in [-127,127],
s = max|w|.  Both integer grids are exactly representable in bf16, so the
conv reduces to an *integer* matmul accumulated in fp32 PSUM — exact —
and runs at full bf16 TensorE rate.  Per (r,s) tap the 3x3 conv is a
128(Cin) x 128(Cout-half) matmul over pixels; 9 taps accumulate in PSUM.
Output rows 0/55 skip the r=0/r=2 taps (they only read zero padding).

Rounding: no rint on any engine, so round-to-nearest-even is done with
the fp32 magic-number trick (v + 1.5*2^23) - 1.5*2^23.

Layout: weights are uploaded pre-transposed [half][Cin, rs, Cout'] so no
PE transposes are needed; BN consts ride one packed [128, 8] transfer.

Schedule: the single modeled DMA bus serializes transfers, so weight
chunks go first (last chunk small so its absmax partial is short), the
first 9 input rows ride the Pool SWDGE ring in between, and the
remaining input bands follow the weights.  PE runs dummy fp32 warm-up
matmuls from ~0.8us so the p-state ramp (0.65 -> 1.2 -> 2.4 GHz) is
done before the first conv matmul.  absmax partials run on Pool, the
BN scale via one ACT Rsqrt, weight-quant pairs split DVE/ACT.  The
epilogue is a 3-engine pipeline (ACT relu+BN, Pool clamp+magic,
DVE unmagic*step) so the end-of-kernel drain chain is short; the final
chunk is staggered 4+4 rows with the last store on the Pool SWDGE ring.
"""

import numpy as np

import concourse.bass as bass
import concourse.mybir as mybir
import concourse.tile as tile
from concourse import bacc, bass_isa
from concourse.bass_utils import run_bass_kernel_spmd

# ---- problem constants (hardcoded per contract) ----
N, C, H, W = 32, 128, 56, 56
O = 256
NCORES = 8
NIMG = N // NCORES  # images per core
HP, WP = H + 2, W + 2  # zero-padded input plane
ROWS_PER_CHUNK = 8
NCHUNK = H // ROWS_PER_CHUNK  # 7
FREE = ROWS_PER_CHUNK * W  # 448 <= 512 (one PSUM bank)

MAGIC = 12582912.0  # 1.5 * 2**23 : fp32 RNE round-to-int trick
QA = 42.5  # 255/6
STEP = float(np.float32(6.0 / 255.0))
BN_EPS = 1e-5

NWARM = 16  # PE warm-up matmuls (p-state ramp) before the first conv

# weight DMA chunks (half, tap0, tap1) over the [C, 9, 128] transposed
# layout.  Each chunk is its own SBUF tile (deps are tile-granular, so
# per-chunk tiles let the absmax partials pipeline behind the DMAs);
# last chunk small so the final absmax partial is short.
W_CHUNKS = [(0, 0, 5), (0, 5, 9), (1, 0, 5), (1, 5, 8), (1, 8, 9)]

f32 = mybir.dt.float32
bf16 = mybir.dt.bfloat16
ALU = mybir.AluOpType
ACTF = mybir.ActivationFunctionType


def _block_rows(b):
    """Quant block b covers unpadded rows: b==0 -> 0..8 (9 rows), b>=1 ->
    8b+1..8b+8 (8 rows).  Conv chunk ch then depends on blocks ch-1, ch
    only (chunk 0 on block 0 only)."""
    if b == 0:
        return 0, 9
    return 8 * b + 1, 8 if b < 6 else 7


def _build_body(tc):
    nc = tc.nc
    xs = nc.dram_tensor("xs", [NIMG, C, H, W], f32, kind="ExternalInput")
    wtT = nc.dram_tensor("wtT", [2, C, 9 * 128], f32, kind="ExternalInput")
    bnc = nc.dram_tensor("bnc", [C, 8], f32, kind="ExternalInput")
    out = nc.dram_tensor("out", [NIMG, O, H, W], f32, kind="ExternalOutput")

    from contextlib import ExitStack

    with ExitStack() as ctx:
        const = ctx.enter_context(tc.tile_pool(name="const", bufs=1))
        wpool = ctx.enter_context(tc.tile_pool(name="wpool", bufs=1))
        xraw = ctx.enter_context(tc.tile_pool(name="xraw", bufs=2))
        xqp = ctx.enter_context(tc.tile_pool(name="xqp", bufs=2))
        tq = ctx.enter_context(tc.tile_pool(name="tq", bufs=3))
        psum = ctx.enter_context(tc.tile_pool(name="psum", bufs=8, space="PSUM"))
        post = ctx.enter_context(tc.tile_pool(name="post", bufs=4))
        outb = ctx.enter_context(tc.tile_pool(name="outp", bufs=3))

        # ACT: dummy Sqrt at t=0 so the (single) activation-table load
        # overlaps the weight DMAs instead of stalling the epilogue stream
        dummy = const.tile([128, 1], f32)
        nc.vector.memset(dummy[:], 1.0)
        nc.scalar.activation(dummy[:], dummy[:], ACTF.Sqrt)

        # PE warm-up source (zeros; fp32 matmuls at 4 cycles/row keep the
        # tensor engine busy through its p-state ramp with few instructions)
        warm = const.tile([128, 128], f32)
        nc.vector.memset(warm[:], 0.0)

        # ============== DMA issue plan ==============
        # Pool SWDGE ring (gen runs on Pool engine, off the shared HWDGE):
        # first 9 input rows + packed BN consts.  SP HWDGE: weight chunks
        # first (they gate absmax -> winv -> all conv), then input bands.
        xr = {}
        xs_flat = [xs.ap()[i].rearrange("c h w -> c (h w)") for i in range(NIMG)]
        xr[0] = xraw.tile([C, H * W], f32, name="xr")
        # rows 0:5 ride the Pool ring early; rows 5:9 queue on SP *after*
        # the weight chunks so they don't push the absmax-critical weight
        # transfers back on the serialized DMA bus
        nc.gpsimd.dma_start(xr[0][:, 0 : 5 * W], xs_flat[0][:, 0 : 5 * W])

        bnct = const.tile([C, 8], f32)
        nc.gpsimd.dma_start(bnct[:], bnc.ap())

        # one tile per weight chunk; wtile[(h, tap)] -> (tile, tap offset)
        wtile = {}
        wchunk = []
        for h, t0, t1 in W_CHUNKS:
            wn = wpool.tile([C, t1 - t0, 128], f32, name=f"wn{h}_{t0}")
            wchunk.append(wn)
            for t in range(t0, t1):
                wtile[(h, t)] = (wn, t - t0)
            nc.sync.dma_start(
                wn[:].rearrange("c r o -> c (r o)"),
                wtT.ap()[h][:, t0 * 128 : t1 * 128],
            )

        nc.sync.dma_start(xr[0][:, 5 * W : 9 * W], xs_flat[0][:, 5 * W : 9 * W])
        nc.sync.dma_start(xr[0][:, 9 * W : 17 * W], xs_flat[0][:, 9 * W : 17 * W])
        nc.sync.dma_start(xr[0][:, 17 * W : 33 * W], xs_flat[0][:, 17 * W : 33 * W])
        nc.sync.dma_start(xr[0][:, 33 * W :], xs_flat[0][:, 33 * W :])

        # ============== PE warm-up ==============
        warm_ps = psum.tile([128, 128], f32, name="warm_ps", bufs=1)
        for _ in range(NWARM):
            nc.tensor.matmul(warm_ps[:], warm[:], warm[:], start=True, stop=True)

        # ============== image-0 quant block 0 + pads ==============
        xq = {}
        xq[0] = xqp.tile([C, HP, WP], bf16, name="xq0")

        def quant_rows(im, r0, nr):
            nf = nr * W
            t1 = tq.tile([C, 9 * W], f32, name="t1")
            # stage A on ACT (relu(QA*x) == clip-low + scale): frees DVE,
            # which carries the absmax partials on the critical path
            nc.scalar.activation(
                t1[:, 0:nf], xr[im][:, r0 * W : r0 * W + nf], ACTF.Relu, scale=QA,
            )
            t2 = tq.tile([C, 9 * W], f32, name="t2")
            nc.vector.tensor_scalar(
                t2[:, 0:nf], t1[:, 0:nf], 255.0, MAGIC, op0=ALU.min, op1=ALU.add,
            )
            nc.vector.tensor_scalar(
                xq[im][:, r0 + 1 : r0 + 1 + nr, 1 : W + 1],
                t2[:, 0:nf].rearrange("c (h w) -> c h w", w=W),
                MAGIC, None, op0=ALU.subtract,
            )

        def quant_block(im, b):
            quant_rows(im, *_block_rows(b))

        def pad_ring(im):
            nc.gpsimd.memset(xq[im][:, 0, :], 0.0)
            nc.gpsimd.memset(xq[im][:, HP - 1, :], 0.0)
            nc.gpsimd.memset(xq[im][:, 1 : HP - 1, 0], 0.0)
            nc.gpsimd.memset(xq[im][:, 1 : HP - 1, WP - 1], 0.0)

        pad_ring(0)
        quant_rows(0, 0, 5)
        quant_rows(0, 5, 4)

        # ============== absmax (DVE partials) -> winv (DVE) ==============
        parts = const.tile([128, len(W_CHUNKS)], f32)
        for pi, wn in enumerate(wchunk):
            nc.vector.tensor_reduce(
                parts[:, pi : pi + 1], wn[:].rearrange("c r o -> c (r o)"),
                axis=mybir.AxisListType.X, op=ALU.max,
                apply_absolute_value=True,
            )
        wabs = const.tile([128, 1], f32)
        nc.vector.tensor_reduce(
            wabs[:], parts[:], axis=mybir.AxisListType.X, op=ALU.max,
        )
        smax = const.tile([C, 1], f32)
        nc.gpsimd.partition_all_reduce(
            smax[:], wabs[:], channels=C, reduce_op=bass_isa.ReduceOp.absmax
        )
        # 1/s via the accurate approx reciprocal (internal Newton passes);
        # winv rel err ~1e-7 flips at most a couple of weight rounding bins
        rscr = const.tile([C, 1], f32)
        srcp = const.tile([C, 1], f32)
        nc.vector.reciprocal_approx_accurate(srcp[:], smax[:], rscr[:])
        winv = const.tile([C, 1], f32)  # 127/s
        nc.vector.tensor_scalar(winv[:], srcp[:], 127.0, None, op0=ALU.mult)

        # ===== BN -> a2/b2 (ACT sqrt + DVE reciprocal + Pool Newton) =====
        veps = const.tile([128, 2], f32)
        nc.gpsimd.tensor_scalar(veps[:], bnct[:, 0:2], BN_EPS, None, op0=ALU.add)
        sv = const.tile([128, 2], f32)
        nc.scalar.activation(sv[:], veps[:], ACTF.Sqrt)
        r_scr = const.tile([128, 2], f32)
        r_cur = const.tile([128, 2], f32)
        nc.vector.reciprocal_approx_accurate(r_cur[:], sv[:], r_scr[:])
        cur = r_cur
        for it in range(2):
            t_sq = const.tile([128, 2], f32, name=f"rs_t{it}")
            nc.gpsimd.tensor_tensor(t_sq[:], cur[:], cur[:], op=ALU.mult)
            t_u = const.tile([128, 2], f32, name=f"rs_u{it}")
            nc.gpsimd.tensor_tensor(t_u[:], veps[:], t_sq[:], op=ALU.mult)
            t_c = const.tile([128, 2], f32, name=f"rs_c{it}")
            nc.gpsimd.tensor_scalar(t_c[:], t_u[:], -0.5, 1.5, op0=ALU.mult, op1=ALU.add)
            t_n = const.tile([128, 2], f32, name=f"rs_n{it}")
            nc.gpsimd.tensor_tensor(t_n[:], cur[:], t_c[:], op=ALU.mult)
            cur = t_n
        bnscale = const.tile([128, 2], f32)
        nc.gpsimd.tensor_tensor(bnscale[:], bnct[:, 2:4], cur[:], op=ALU.mult)
        # b2 = 42.5 * (beta - mean*bnscale)
        msc = const.tile([128, 2], f32)
        nc.gpsimd.tensor_tensor(msc[:], bnct[:, 6:8], bnscale[:], op=ALU.mult)
        bmm = const.tile([128, 2], f32)
        nc.gpsimd.tensor_tensor(bmm[:], bnct[:, 4:6], msc[:], op=ALU.subtract)
        b2 = const.tile([128, 2], f32)
        nc.gpsimd.tensor_scalar(b2[:], bmm[:], QA, None, op0=ALU.mult)
        # a2 = bnscale * s/127   (42.5 * 6/255 == 1)
        qs2 = const.tile([128, 1], f32)
        nc.gpsimd.tensor_scalar(qs2[:], smax[:], 1.0 / 127.0, None, op0=ALU.mult)
        a2 = const.tile([128, 2], f32)
        nc.gpsimd.tensor_scalar(a2[:], bnscale[:], qs2[:], None, op0=ALU.mult)

        # ============== weight quant (feeds conv tap-by-tap) ==============
        wq = [wpool.tile([C, 9, 128], bf16, name=f"wq{h}") for h in range(2)]

        def wquant(h, taps, on_act=False):
            """taps must be contiguous and within one DMA chunk tile."""
            n = len(taps)
            wn, off = wtile[(h, taps[0])]
            wtmp = tq.tile([C, 5, 128], f32, name="wtmp")
            src = wn[:, off : off + n, :]
            dst = wq[h][:, taps[0] : taps[0] + n, :]
            if on_act:
                nc.scalar.activation(
                    wtmp[:, 0:n, :], src, ACTF.Copy, bias=MAGIC, scale=winv[:]
                )
                nc.scalar.activation(dst, wtmp[:, 0:n, :], ACTF.Copy, bias=-MAGIC)
            else:
                nc.vector.tensor_scalar(
                    wtmp[:, 0:n, :], src, winv[:], MAGIC, op0=ALU.mult, op1=ALU.add,
                )
                nc.vector.tensor_scalar(
                    dst, wtmp[:, 0:n, :], MAGIC, None, op0=ALU.subtract
                )

        # half-0, in tap consumption order (r=1 taps 3,4,5 first), fed as
        # singles just-in-time; the last triple on ACT, otherwise idle here
        wquant(0, (3,))
        wquant(0, (4,))
        wquant(0, (5,))
        wquant(0, (0,))
        wquant(0, (1, 2))
        wquant(0, (6, 7, 8), on_act=True)

        quant_block(0, 1)

        # half-1 (needed ~12us in): DVE, interleaved after image-0 blocks
        wquant(1, (3, 4))
        wquant(1, (5,))
        quant_block(0, 2)
        wquant(1, (0, 1, 2))
        wquant(1, (6, 7))
        wquant(1, (8,))
        for b in range(3, NCHUNK):
            quant_block(0, b)

        # ===================== conv + epilogue main loop ====================
        BAND_OF = [0, 0, 0, 0, 1, 1, 1]
        BAND_COLS = [4 * FREE, 3 * FREE]
        BAND_OFF = [0, 4 * FREE]

        # taps ordered r=1 first so the start=True tap always covers the
        # full output range; r=0 / r=2 taps are trimmed at the image edges
        # (they only read zero padding there)
        TAP_ORDER = [(1, 0), (1, 1), (1, 2), (0, 0), (0, 1), (0, 2), (2, 0), (2, 1), (2, 2)]

        def chunk_matmuls(im, half, ch, nrw=ROWS_PER_CHUNK, ro=0):
            ps = psum.tile([128, nrw * W], f32, name="ps", bufs=5)
            rb = ch * ROWS_PER_CHUNK + ro  # first output row of the piece
            for i, (r, s) in enumerate(TAP_ORDER):
                o0, o1 = 0, nrw  # output row range within the piece
                if r == 0 and rb == 0:
                    o0 = 1  # output row 0: r=0 reads only top padding
                if r == 2 and rb + nrw == H:
                    o1 = nrw - 1  # output row 55: r=2 reads only bottom padding
                nc.tensor.matmul(
                    ps[:, o0 * W : o1 * W],
                    wq[half][:, r * 3 + s, :],
                    xq[im][:, rb + r + o0 : rb + r + o1, s : s + W],
                    start=(i == 0),
                    stop=(i == 8),
                )
            return ps

        def epilogue(im, half, ch, ps, ob, nrw=ROWS_PER_CHUNK, ro=0, boff=None, no_pool=False):
            """3-engine epilogue: ACT relu+BN -> Pool clamp+magic -> DVE
            unmagic*step into the store buffer.  no_pool keeps the chain on
            ACT+DVE only (shorter latency for the kernel tail)."""
            nf = nrw * W
            if boff is None:
                band = BAND_OF[ch]
                boff = (ch - (0 if band == 0 else 4)) * FREE + ro * W
            tpost = post.tile([128, FREE], f32, name="tpost")
            nc.scalar.activation(
                tpost[:, 0:nf], ps[:], ACTF.Relu,
                bias=b2[:, half : half + 1], scale=a2[:, half : half + 1],
            )
            u = post.tile([128, FREE], f32, name="u")
            clamp_eng = nc.vector if no_pool else nc.gpsimd
            clamp_eng.tensor_scalar(
                u[:, 0:nf], tpost[:, 0:nf], 255.0, MAGIC, op0=ALU.min, op1=ALU.add,
            )
            nc.vector.tensor_scalar(
                ob[:, boff : boff + nf], u[:, 0:nf], MAGIC, STEP,
                op0=ALU.subtract, op1=ALU.mult,
            )
            return boff

        def store_band(im, half, band, ob, eng):
            eng.dma_start(
                out.ap()[im, half * 128 : (half + 1) * 128]
                .rearrange("o h w -> o (h w)")[
                    :, BAND_OFF[band] : BAND_OFF[band] + BAND_COLS[band]
                ],
                ob[:, 0 : BAND_COLS[band]],
            )

        def store_piece(im, half, band, ob, boff, nf, eng=None, src_off=None):
            if src_off is None:
                src_off = boff
            (eng or nc.sync).dma_start(
                out.ap()[im, half * 128 : (half + 1) * 128]
                .rearrange("o h w -> o (h w)")[
                    :, BAND_OFF[band] + boff : BAND_OFF[band] + boff + nf
                ],
                ob[:, src_off : src_off + nf],
            )

        for im in range(NIMG):
            last_im = im == NIMG - 1
            if im + 1 < NIMG:
                # prefetch next image (2 band DMAs on SP)
                xr[im + 1] = xraw.tile([C, H * W], f32, name="xr")
                nc.sync.dma_start(
                    xr[im + 1][:, 0 : 33 * W], xs_flat[im + 1][:, 0 : 33 * W]
                )
                nc.sync.dma_start(
                    xr[im + 1][:, 33 * W :], xs_flat[im + 1][:, 33 * W :]
                )

            for half in range(2):
                final_grp = last_im and half == 1
                if final_grp:
                    # drain-friendly tail: per-chunk dedicated tiles stored
                    # as soon as written; Pool stays OFF the epilogue (its
                    # SWDGE gens + slow gpsimd ops congested the drain) —
                    # ACT+DVE only, stores on SP, except the very last
                    # 3-row piece which rides the (now idle) Pool ring
                    for ch in range(NCHUNK):
                        obF = outb.tile([128, FREE], f32, name="obF")
                        if ch == NCHUNK - 1:
                            # 5-row piece first (stores via the Pool ring
                            # while the 3-row piece computes); the small
                            # 3-row piece is the true tail on SP
                            for ro, nrw in [(0, 5), (5, 3)]:
                                last = ro == 5
                                ps = chunk_matmuls(im, half, ch, nrw=nrw, ro=ro)
                                epilogue(
                                    im, half, ch, ps, obF, nrw=nrw, ro=ro,
                                    boff=ro * W, no_pool=True,
                                )
                                store_piece(
                                    im, half, 1, obF, 2 * FREE + ro * W, nrw * W,
                                    eng=nc.sync if last else nc.gpsimd,
                                    src_off=ro * W,
                                )
                        else:
                            ps = chunk_matmuls(im, half, ch)
                            epilogue(im, half, ch, ps, obF, boff=0, no_pool=True)
                            band = BAND_OF[ch]
                            store_piece(
                                im, half, band, obF,
                                (ch - (0 if band == 0 else 4)) * FREE, FREE,
                                eng=nc.sync,
                                src_off=0,
                            )
                else:
                    obA = outb.tile([128, 4 * FREE], f32, name="ob")
                    obB = outb.tile([128, 4 * FREE], f32, name="ob")
                    for ch in range(NCHUNK):
                        band = BAND_OF[ch]
                        ob = obA if band == 0 else obB
                        ps = chunk_matmuls(im, half, ch)
                        epilogue(im, half, ch, ps, ob)
                        if ch == 3:
                            store_band(im, half, 0, obA, nc.gpsimd)
                        elif ch == NCHUNK - 1:
                            store_band(im, half, 1, obB, nc.gpsimd)

                if half == 0 and im + 1 < NIMG:
                    # next image's quant chain between halves
                    xq[im + 1] = xqp.tile([C, HP, WP], bf16, name=f"xq{im + 1}")
                    pad_ring(im + 1)
                    for b in range(NCHUNK):
                        quant_block(im + 1, b)


_CACHED = None


def _get_program():
    global _CACHED
    if _CACHED is None:
        nc = bacc.Bacc(
            "TRN2", target_bir_lowering=False, debug=False, num_devices=NCORES
        )
        with tile.TileContext(nc) as tc:
            _build_body(tc)
        nc.compile()
        _CACHED = nc
    return _CACHED


def run_on_cores(inputs, trace=False, **kw):
    """Run the SPMD kernel; returns (full_output, BassKernelResults)."""
    nc = _get_program()
    x = np.ascontiguousarray(inputs["x"], dtype=np.float32)
    w = np.asarray(inputs["weight"], dtype=np.float32)
    # pre-transposed weight layout [half][Cin, (rs, Cout')]
    wT = w.transpose(1, 2, 3, 0).reshape(C, 9, O)
    wtT = np.stack(
        [
            np.ascontiguousarray(wT[:, :, :128].reshape(C, 9 * 128)),
            np.ascontiguousarray(wT[:, :, 128:].reshape(C, 9 * 128)),
        ]
    )
    # packed BN consts [128, 8]: var | gamma | beta | mean (2 cols each,
    # channel h*128+p -> row p, col pair h)
    bnc = np.concatenate(
        [
            np.asarray(inputs[k], dtype=np.float32).reshape(2, 128).T
            for k in ("var", "gamma", "beta", "mean")
        ],
        axis=1,
    )
    bnc = np.ascontiguousarray(bnc)
    in_maps = []
    for c in range(NCORES):
        in_maps.append(
            {
                "xs": np.ascontiguousarray(x[c * NIMG : (c + 1) * NIMG]),
                "wtT": wtT,
                "bnc": bnc,
            }
        )
    res = run_bass_kernel_spmd(nc, in_maps, list(range(NCORES)), trace=trace, **kw)
    full = np.concatenate([res.results[c]["out"] for c in range(NCORES)], axis=0)
    return full.astype(np.float32), res


def kernel(**inputs) -> np.ndarray:
    full, _ = run_on_cores(inputs)
    return full
